# revision 9
# baseline (speedup 1.0000x reference)
"""Multi-head attention encoder kernel for Trainium2 (8 NeuronCores).

Problem: B=8, C=3, S=1024, DIM=768, H=3, HD=256.
  x = linear_embed.reshape(B,C,S,H,HD)
  q/k/v = per-head Linear(x) ; scores = q@k^T/sqrt(HD) ; attn = softmax
  out = attn@v -> [B,C,S,DIM] -> transpose -> [B,S,C*DIM]

Sharding: data-parallel over batch B across the 8 cores (weights
replicated).  Each core handles all C*H = 9 attention heads of its batch
element.

Per-core layout strategy (per (c,h) pair):
  xT  [d,s]  = PE-transpose of x slice            (d on partitions)
  qT  [e,s]  = WqT.T @ xT  (+bq per-partition)    (e on partitions)
  kT  [e,s]  = WkT.T @ xT  (+bk per-partition)
  v   [t,e]  = xT.T @ WvT  (+bv broadcast), extended with a ones column
  sT  [t,s]  = kT.T @ qT   -> exp(sT/16) on scalar engine = pT
  o   [s,e+1]= pT.T @ v_ext : col HD is the softmax denominator
  out = o[:, :HD] * recip(o[:, HD])               (denominator trick:
        softmax rows sum to 1, so the v bias passes through exactly and
        max-subtraction cancels; scores have |x| < ~3 so exp is safe)
"""

import contextlib
import numpy as np

import concourse.bass as bass
import concourse.tile as tile
from concourse import bacc, mybir
from concourse import bass_utils
from concourse.masks import make_identity

B, C, S, DIM, H = 8, 3, 1024, 768, 3
HD = DIM // H          # 256
P = 128                # partitions
NS = S // P            # 8 s-tiles (and t-tiles)
SCALE = 1.0 / 16.0     # 1/sqrt(HD)
F32 = mybir.dt.float32

# float32r runs the PE at 1 row/cycle (vs 4 for float32) when N>=256.
# The BIR verifier requires fp32r matmul inputs to be *produced* as fp32r
# (rounded), so every tile that feeds a matmul is declared float32r and the
# PSUM->SBUF evacuation op does the rounding.
F32R = mybir.dt.float32r


def _emit_pair_stage_a(tc, st, x, c, h):
    """Transposes + projections + scores/exp for one (c,h) pair.

    Returns the tiles stage B (PV + epilogue) needs.
    """
    nc = tc.nc

    # ---- load x slice [S, HD] for this (c,h) as 8 s-tiles, packed in one
    # SBUF tile [128, NS*HD] (column block n holds s-tile n).
    x_ch = st.xpool.tile([P, NS * HD], F32, tag="xch", name="xch")
    nc.sync.dma_start(
        x_ch[:].rearrange("p (n d) -> p n d", d=HD),
        x[c, :, h * HD:(h + 1) * HD].rearrange("(n p) d -> p n d", p=P),
    )

    # ---- xT [d, s]: 2 partition tiles of [128, S]
    xT = [st.work.tile([P, S], F32R, tag=f"xT{j}", name=f"xT{j}") for j in range(2)]
    for j in range(2):
        for half in range(2):
            ps = st.ps_tr.tile([P, 512], F32, tag="tr", name="ps_tr_t")
            for k in range(4):
                si = half * 4 + k
                blk = x_ch[:, si * HD + j * P: si * HD + (j + 1) * P]
                nc.tensor.transpose(ps[:, k * P:(k + 1) * P], blk, st.ident[:])
            nc.vector.tensor_copy(xT[j][:, half * 512:(half + 1) * 512], ps[:])

    # ---- qT / kT [e, s]
    qT = [st.work.tile([P, S], F32R, tag=f"qT{i}", name=f"qT{i}") for i in range(2)]
    kT = [st.work.tile([P, S], F32R, tag=f"kT{i}", name=f"kT{i}") for i in range(2)]
    for name, dest in (("q", qT), ("k", kT)):
        wt = st.wT[name, h]
        for i in range(2):
            pss = [st.ps_proj.tile([P, 512], F32, tag="proj", name="ps_proj_qk") for _ in range(2)]
            for j in range(2):
                for half in range(2):
                    nc.tensor.matmul(
                        pss[half][:],
                        (wt[j][:, i * P:(i + 1) * P]),
                        (xT[j][:, half * 512:(half + 1) * 512]),
                        start=(j == 0),
                        stop=(j == 1),
                    )
            for half in range(2):
                dslice = dest[i][:, half * 512:(half + 1) * 512]
                if name == "q":
                    nc.vector.tensor_scalar_add(dslice, pss[half][:], st.bias["q", h][i][:])
                else:
                    nc.scalar.activation(
                        dslice, pss[half][:],
                        mybir.ActivationFunctionType.Identity,
                        bias=st.bias["k", h][i][:],
                    )

    # ---- v_ext [t, HD+2] per t-tile (col HD = 1.0 for the denominator,
    # col HD+1 = 0.0 pad: fp32r matmuls need an even moving free dim)
    v_ext = [st.vpool.tile([P, HD + 2], F32R, tag="v", name="v_ext") for _ in range(NS)]
    for ti in range(NS):
        ps = st.ps_proj.tile([P, HD], F32, tag="proj", name="ps_proj_v")
        for j in range(2):
            nc.tensor.matmul(
                ps[:],
                (xT[j][:, ti * P:(ti + 1) * P]),
                (st.wT["v", h][j][:]),
                start=(j == 0),
                stop=(j == 1),
            )
        nc.vector.tensor_add(v_ext[ti][:, 0:HD], ps[:], st.bvb[h][:])
        nc.vector.tensor_copy(v_ext[ti][:, HD:HD + 2], st.one_zero[:])

    # ---- scoresT -> exp -> pT [t, s]
    pT = [st.ppool.tile([P, S], F32R, tag="pT", name="pT") for _ in range(NS)]
    for ti in range(NS):
        for half in range(2):
            ps = st.ps_s.tile([P, 512], F32, tag="s", name="ps_s_t")
            for i in range(2):
                nc.tensor.matmul(
                    ps[:],
                    (kT[i][:, ti * P:(ti + 1) * P]),
                    (qT[i][:, half * 512:(half + 1) * 512]),
                    start=(i == 0),
                    stop=(i == 1),
                )
            nc.scalar.activation(
                pT[ti][:, half * 512:(half + 1) * 512], ps[:],
                mybir.ActivationFunctionType.Exp, scale=SCALE,
            )

    return pT, v_ext


def _emit_pair_stage_b(tc, st, out, c, h, pT, v_ext):
    """PV accumulation + epilogue + output DMA for one (c,h) pair."""
    nc = tc.nc
    for si in range(NS):
        ps = st.ps_o.tile([P, HD + 2], F32, tag="o", name="ps_o_t")
        for ti in range(NS):
            nc.tensor.matmul(
                ps[:],
                (pT[ti][:, si * P:(si + 1) * P]),
                (v_ext[ti][:]),
                start=(ti == 0),
                stop=(ti == NS - 1),
            )
        rec = st.opool.tile([P, 1], F32, tag="rec", name="rec")
        nc.vector.reciprocal(rec[:], ps[:, HD:HD + 1])
        o_sb = st.opool.tile([P, HD], F32, tag="osb", name="osb")
        nc.vector.tensor_scalar_mul(o_sb[:], ps[:, 0:HD], rec[:])
        nc.sync.dma_start(
            out[si * P:(si + 1) * P, c * DIM + h * HD: c * DIM + (h + 1) * HD],
            o_sb[:],
        )


class _State:
    pass


def _kernel_body(ctx, tc, out, x, w_aps, b_aps):
    nc = tc.nc
    st = _State()

    st.consts = ctx.enter_context(tc.tile_pool(name="consts", bufs=1))
    st.prep = ctx.enter_context(tc.tile_pool(name="prep", bufs=2))
    st.xpool = ctx.enter_context(tc.tile_pool(name="xpool", bufs=2))
    st.work = ctx.enter_context(tc.tile_pool(name="work", bufs=2))
    st.vpool = ctx.enter_context(tc.tile_pool(name="vpool", bufs=2 * NS))
    st.ppool = ctx.enter_context(tc.tile_pool(name="ppool", bufs=2 * NS))
    st.opool = ctx.enter_context(tc.tile_pool(name="opool", bufs=4))
    st.ps_tr = ctx.enter_context(tc.tile_pool(name="ps_tr", bufs=1, space=bass.MemorySpace.PSUM))
    st.ps_proj = ctx.enter_context(tc.tile_pool(name="ps_proj", bufs=3, space=bass.MemorySpace.PSUM))
    st.ps_s = ctx.enter_context(tc.tile_pool(name="ps_s", bufs=2, space=bass.MemorySpace.PSUM))
    st.ps_o = ctx.enter_context(tc.tile_pool(name="ps_o", bufs=2, space=bass.MemorySpace.PSUM))

    st.ident = st.consts.tile([P, P], F32)
    make_identity(nc, st.ident)

    st.one_zero = st.consts.tile([P, 2], F32)
    nc.gpsimd.memset(st.one_zero[:, 0:1], 1.0)
    nc.gpsimd.memset(st.one_zero[:, 1:2], 0.0)

    # ---- weights: transpose W[h] ([e,d] in DRAM) into wT tiles [d,e]
    # (2 partition tiles of [128, HD] per matrix per head).
    st.wT = {}
    st.bias = {}
    st.bvb = {}
    for name in ("q", "k", "v"):
        w_ap, b_ap = w_aps[name], b_aps[name]
        for h in range(H):
            raw = []
            for i in range(2):
                r = st.prep.tile([P, HD], F32, tag="wraw", name="wraw")
                nc.sync.dma_start(r[:], w_ap[h, i * P:(i + 1) * P, :])
                raw.append(r)
            wt = [st.consts.tile([P, HD], F32R, tag=f"wT_{name}{h}{j}", name=f"wT_{name}{h}{j}") for j in range(2)]
            for j in range(2):
                ps = st.ps_tr.tile([P, HD], F32, tag="tr", name="ps_tr_w")
                for i in range(2):
                    nc.tensor.transpose(
                        ps[:, i * P:(i + 1) * P], raw[i][:, j * P:(j + 1) * P],
                        st.ident[:],
                    )
                nc.vector.tensor_copy(wt[j][:], ps[:])
            st.wT[name, h] = wt

            if name in ("q", "k"):
                bt = []
                for i in range(2):
                    t = st.consts.tile([P, 1], F32, tag=f"b_{name}{h}{i}", name=f"b_{name}{h}{i}")
                    nc.sync.dma_start(
                        t[:],
                        b_ap[h, i * P:(i + 1) * P].rearrange("(p f) -> p f", f=1),
                    )
                    bt.append(t)
                st.bias[name, h] = bt
            else:
                row = st.prep.tile([1, HD], F32, tag="bvrow", name="bvrow")
                nc.sync.dma_start(row[:], b_ap[h].rearrange("(p f) -> p f", p=1))
                bb = st.consts.tile([P, HD], F32, tag=f"bvb{h}", name=f"bvb{h}")
                nc.gpsimd.partition_broadcast(bb[:], row[:])
                st.bvb[h] = bb

    # ---- main loop, software-pipelined: stage A(i) emitted before
    # stage B(i-1) so the PE has independent matmul work while the
    # scalar engine finishes pair i-1's exp evacuations.
    pairs = [(c, h) for c in range(C) for h in range(H)]
    pending = None
    for (c, h) in pairs:
        ab = _emit_pair_stage_a(tc, st, x, c, h)
        if pending is not None:
            pc, ph, ppT, pv = pending
            _emit_pair_stage_b(tc, st, out, pc, ph, ppT, pv)
        pending = (c, h, ab[0], ab[1])
    pc, ph, ppT, pv = pending
    _emit_pair_stage_b(tc, st, out, pc, ph, ppT, pv)


def build_module():
    nc = bacc.Bacc("TRN2", target_bir_lowering=False, debug=False, num_devices=B)
    x = nc.dram_tensor("x", (C, S, DIM), F32, kind="ExternalInput").ap()
    w_aps, b_aps = {}, {}
    for name in ("q", "k", "v"):
        w_aps[name] = nc.dram_tensor(f"w{name}", (H, HD, HD), F32, kind="ExternalInput").ap()
        b_aps[name] = nc.dram_tensor(f"b{name}", (H, HD), F32, kind="ExternalInput").ap()
    out = nc.dram_tensor("out", (S, C * DIM), F32, kind="ExternalOutput").ap()

    with tile.TileContext(nc) as tc:
        with contextlib.ExitStack() as ctx:
            _kernel_body(ctx, tc, out, x, w_aps, b_aps)
    nc.compile()
    return nc


def run(inputs, trace=False, **kw):
    le = np.asarray(inputs["linear_embed"], dtype=np.float32)
    nc = build_module()
    in_maps = []
    for b in range(B):
        m = {"x": np.ascontiguousarray(le[b])}
        for name in ("q", "k", "v"):
            m[f"w{name}"] = np.asarray(inputs[f"W{name}"], dtype=np.float32)
            m[f"b{name}"] = np.asarray(inputs[f"b{name}"], dtype=np.float32)
        in_maps.append(m)
    res = bass_utils.run_bass_kernel_spmd(
        nc, in_maps, core_ids=list(range(B)), trace=trace, **kw
    )
    out = np.stack([res.results[b]["out"] for b in range(B)], axis=0)
    return out, res


def kernel(**inputs) -> np.ndarray:
    out, _ = run(inputs)
    return out


# revision 14
# speedup vs baseline: 1.1473x; 1.1473x over previous
"""Multi-head attention encoder kernel for Trainium2 (8 NeuronCores).

Problem: B=8, C=3, S=1024, DIM=768, H=3, HD=256.
  x = linear_embed.reshape(B,C,S,H,HD)
  q/k/v = per-head Linear(x) ; scores = q@k^T/sqrt(HD) ; attn = softmax
  out = attn@v -> [B,C,S,DIM] -> transpose -> [B,S,C*DIM]

Sharding: data-parallel over batch B across the 8 cores (weights
replicated).  Each core handles all C*H = 9 attention heads of its batch
element.

Per-core layout strategy (per (c,h) pair):
  xT  [d,s]  = PE-transpose of x slice            (d on partitions)
  qT  [e,s]  = WqT.T @ xT  (+bq per-partition)    (e on partitions)
  kT  [e,s]  = WkT.T @ xT  (+bk per-partition)
  v   [t,e]  = xT.T @ WvT  (+bv broadcast), extended with a [1,0] column
               pair (ones column -> softmax denominator; fp32r needs an
               even moving free dim)
  sT  [t,s]  = kT.T @ qT   -> exp(sT/16) on scalar engine = pT
  o   [s,e+2]= pT.T @ v_ext : col HD is the softmax denominator
  out = o[:, :HD] * recip(o[:, HD])               (denominator trick:
        softmax rows sum to 1, so the v bias passes through exactly and
        max-subtraction cancels; scores have |x| < ~3 so exp is safe)

All matmul inputs are float32r (PE runs 1 row/cycle vs 4 for float32 when
N>=256); the PSUM->SBUF evacuation ops perform the required fp32r
rounding.  The x transposes stay float32 (fp32r transposes hang the HW).

Scheduling: the emission order software-pipelines three pairs: while pair
i's scores stream on the PE (whose exp evacuations pace the scalar
engine), the in-order PE also gets pair i-1's PV groups and pair i+1's
x transposes as independent work, so it never stalls on a single
dependency chain.
"""

import contextlib
import numpy as np

import concourse.bass as bass
import concourse.tile as tile
from concourse import bacc, mybir
from concourse import bass_utils
from concourse.masks import make_identity

B, C, S, DIM, H = 8, 3, 1024, 768, 3
HD = DIM // H          # 256
P = 128                # partitions
NS = S // P            # 8 s-tiles (and t-tiles)
SCALE = 1.0 / 16.0     # 1/sqrt(HD)
F32 = mybir.dt.float32
F32R = mybir.dt.float32r


class _State:
    pass


def _emit_x_dma(tc, st, x, c, h):
    """Prefetch the [S, HD] x slice for (c,h): 8 s-tiles packed in one
    [128, NS*HD] SBUF tile (column block n holds s-tile n)."""
    nc = tc.nc
    x_ch = st.xpool.tile([P, NS * HD], F32, tag="xch", name="xch")
    nc.sync.dma_start(
        x_ch[:].rearrange("p (n d) -> p n d", d=HD),
        x[c, :, h * HD:(h + 1) * HD].rearrange("(n p) d -> p n d", p=P),
    )
    return x_ch


def _new_xT(st):
    return [st.work.tile([P, S], F32R, tag=f"xT{j}", name=f"xT{j}") for j in range(2)]


def _emit_tr_group(tc, st, x_ch, g, xT):
    """One transpose group: 4 PE transposes of [128,128] x blocks into a
    [128,512] PSUM tile, evacuated into xT[j][:, half].  g in 0..3 maps to
    (j, half)."""
    nc = tc.nc
    j, half = divmod(g, 2)
    ps = st.ps_shared.tile([P, 512], F32, tag="shared", name="ps_tr_t")
    for k in range(4):
        si = half * 4 + k
        blk = x_ch[:, si * HD + j * P: si * HD + (j + 1) * P]
        nc.tensor.transpose(ps[:, k * P:(k + 1) * P], blk, st.ident[:])
    nc.vector.tensor_copy(xT[j][:, half * 512:(half + 1) * 512], ps[:])


def _emit_qk(tc, st, h, xT):
    nc = tc.nc
    qT = [st.work.tile([P, S], F32R, tag=f"qT{i}", name=f"qT{i}") for i in range(2)]
    kT = [st.work.tile([P, S], F32R, tag=f"kT{i}", name=f"kT{i}") for i in range(2)]
    for name, dest in (("q", qT), ("k", kT)):
        wt = st.wT[name, h]
        for i in range(2):
            pss = [st.ps_proj.tile([P, 512], F32, tag="proj", name="ps_proj_qk")
                   for _ in range(2)]
            for j in range(2):
                for half in range(2):
                    nc.tensor.matmul(
                        pss[half][:],
                        wt[j][:, i * P:(i + 1) * P],
                        xT[j][:, half * 512:(half + 1) * 512],
                        start=(j == 0),
                        stop=(j == 1),
                    )
            for half in range(2):
                nc.vector.tensor_scalar_add(
                    dest[i][:, half * 512:(half + 1) * 512],
                    pss[half][:], st.bias[name, h][i][:])
    return qT, kT


def _emit_v_tile(tc, st, h, xT, v_ext, ti):
    nc = tc.nc
    ps_v = st.ps_proj.tile([P, HD + 2], F32, tag="proj", name="ps_proj_v")
    for j in range(2):
        nc.tensor.matmul(
            ps_v[:],
            xT[j][:, ti * P:(ti + 1) * P],
            st.wT["v", h][j][:],
            start=(j == 0),
            stop=(j == 1),
        )
    nc.vector.tensor_add(v_ext[ti][:], ps_v[:], st.bvb[h][:])


def _emit_scores_tile(tc, st, qT, kT, pT, ti):
    nc = tc.nc
    for half in range(2):
        ps = st.ps_s.tile([P, 512], F32, tag="s", name="ps_s_t")
        for i in range(2):
            nc.tensor.matmul(
                ps[:],
                kT[i][:, ti * P:(ti + 1) * P],
                qT[i][:, half * 512:(half + 1) * 512],
                start=(i == 0),
                stop=(i == 1),
            )
        nc.scalar.activation(
            pT[ti][:, half * 512:(half + 1) * 512], ps[:],
            mybir.ActivationFunctionType.Exp, scale=SCALE,
        )


def _emit_pv_group(tc, st, out, c, h, pT, v_ext, si):
    """One PV accumulation group + epilogue + output DMA."""
    nc = tc.nc
    ps = st.ps_shared.tile([P, HD + 2], F32, tag="shared", name="ps_o_t")
    for ti in range(NS):
        nc.tensor.matmul(
            ps[:],
            pT[ti][:, si * P:(si + 1) * P],
            v_ext[ti][:],
            start=(ti == 0),
            stop=(ti == NS - 1),
        )
    rec = st.opool.tile([P, 1], F32, tag="rec", name="rec")
    nc.vector.reciprocal(rec[:], ps[:, HD:HD + 1])
    o_sb = st.opool.tile([P, HD], F32, tag="osb", name="osb")
    if si % 2 == 0:
        nc.scalar.activation(o_sb[:], ps[:, 0:HD],
                             mybir.ActivationFunctionType.Identity, scale=rec[:])
    else:
        nc.vector.tensor_scalar_mul(o_sb[:], ps[:, 0:HD], rec[:])
    nc.sync.dma_start(
        out[si * P:(si + 1) * P, c * DIM + h * HD: c * DIM + (h + 1) * HD],
        o_sb[:],
    )


def _emit_weight_prep(tc, st, w_aps, b_aps):
    nc = tc.nc
    st.ident = st.consts.tile([P, P], F32)
    make_identity(nc, st.ident)

    zeros2 = st.consts.tile([P, 2], F32)
    nc.gpsimd.memset(zeros2[:], 0.0)

    st.wT = {}
    st.bias = {}
    st.bvb = {}
    for name in ("q", "k", "v"):
        w_ap, b_ap = w_aps[name], b_aps[name]
        for h in range(H):
            raw = []
            for i in range(2):
                r = st.prep.tile([P, HD], F32, tag="wraw", name="wraw")
                nc.sync.dma_start(r[:], w_ap[h, i * P:(i + 1) * P, :])
                raw.append(r)
            wcols = HD + 2 if name == "v" else HD
            wt = [st.consts.tile([P, wcols], F32R, tag=f"wT_{name}{h}{j}",
                                 name=f"wT_{name}{h}{j}") for j in range(2)]
            for j in range(2):
                ps = st.ps_shared.tile([P, HD], F32, tag="shared", name="ps_tr_w")
                for i in range(2):
                    nc.tensor.transpose(
                        ps[:, i * P:(i + 1) * P], raw[i][:, j * P:(j + 1) * P],
                        st.ident[:],
                    )
                nc.vector.tensor_copy(wt[j][:, 0:HD], ps[:])
                if name == "v":
                    nc.vector.tensor_copy(wt[j][:, HD:HD + 2], zeros2[:])
            st.wT[name, h] = wt

            if name in ("q", "k"):
                bt = []
                for i in range(2):
                    t = st.consts.tile([P, 1], F32, tag=f"b_{name}{h}{i}",
                                       name=f"b_{name}{h}{i}")
                    nc.sync.dma_start(
                        t[:],
                        b_ap[h, i * P:(i + 1) * P].rearrange("(p f) -> p f", f=1),
                    )
                    bt.append(t)
                st.bias[name, h] = bt
            else:
                row = st.prep.tile([1, HD], F32, tag="bvrow", name="bvrow")
                nc.sync.dma_start(row[:], b_ap[h].rearrange("(p f) -> p f", p=1))
                bb = st.consts.tile([P, HD + 2], F32, tag=f"bvb{h}", name=f"bvb{h}")
                nc.gpsimd.partition_broadcast(bb[:, 0:HD], row[:])
                nc.gpsimd.memset(bb[:, HD:HD + 1], 1.0)
                nc.gpsimd.memset(bb[:, HD + 1:HD + 2], 0.0)
                st.bvb[h] = bb


def _kernel_body(ctx, tc, out, x, w_aps, b_aps):
    st = _State()

    st.consts = ctx.enter_context(tc.tile_pool(name="consts", bufs=1))
    st.prep = ctx.enter_context(tc.tile_pool(name="prep", bufs=6))
    st.xpool = ctx.enter_context(tc.tile_pool(name="xpool", bufs=3))
    st.work = ctx.enter_context(tc.tile_pool(name="work", bufs=2))
    st.vpool = ctx.enter_context(tc.tile_pool(name="vpool", bufs=2 * NS))
    st.ppool = ctx.enter_context(tc.tile_pool(name="ppool", bufs=2 * NS))
    st.opool = ctx.enter_context(tc.tile_pool(name="opool", bufs=6))
    st.ps_shared = ctx.enter_context(
        tc.tile_pool(name="ps_shared", bufs=3, space=bass.MemorySpace.PSUM))
    st.ps_proj = ctx.enter_context(
        tc.tile_pool(name="ps_proj", bufs=3, space=bass.MemorySpace.PSUM))
    st.ps_s = ctx.enter_context(
        tc.tile_pool(name="ps_s", bufs=2, space=bass.MemorySpace.PSUM))

    pairs = [(c, h) for c in range(C) for h in range(H)]
    n = len(pairs)

    # x prefetches for the first two pairs go first so the DMA engines fill
    # SBUF while the PE chews through the weight transposes.
    x_ch = {0: _emit_x_dma(tc, st, x, *pairs[0])}
    x_ch[1] = _emit_x_dma(tc, st, x, *pairs[1])

    _emit_weight_prep(tc, st, w_aps, b_aps)

    # pair 0's transposes have nothing to hide under; emit them standalone
    xT = {0: _new_xT(st)}
    for g in range(4):
        _emit_tr_group(tc, st, x_ch[0], g, xT[0])

    pending = None  # (c, h, pT, v_ext) of the previous pair
    for idx, (c, h) in enumerate(pairs):
        if idx + 2 < n:
            x_ch[idx + 2] = _emit_x_dma(tc, st, x, *pairs[idx + 2])
        qT, kT = _emit_qk(tc, st, h, xT[idx])
        if idx + 1 < n:
            xT[idx + 1] = _new_xT(st)

        v_ext = [st.vpool.tile([P, HD + 2], F32R, tag="v", name="v_ext")
                 for _ in range(NS)]
        pT = [st.ppool.tile([P, S], F32R, tag="pT", name="pT") for _ in range(NS)]
        for ti in range(NS):
            _emit_v_tile(tc, st, h, xT[idx], v_ext, ti)
            _emit_scores_tile(tc, st, qT, kT, pT, ti)
            if pending is not None:
                _emit_pv_group(tc, st, out, pending[0], pending[1],
                               pending[2], pending[3], ti)
            if ti % 2 == 1 and idx + 1 < n:
                _emit_tr_group(tc, st, x_ch[idx + 1], (ti - 1) // 2, xT[idx + 1])
        del x_ch[idx]
        pending = (c, h, pT, v_ext)

    pc, ph, ppT, pv = pending
    for si in range(NS):
        _emit_pv_group(tc, st, out, pc, ph, ppT, pv, si)


def build_module():
    nc = bacc.Bacc("TRN2", target_bir_lowering=False, debug=False, num_devices=B)
    x = nc.dram_tensor("x", (C, S, DIM), F32, kind="ExternalInput").ap()
    w_aps, b_aps = {}, {}
    for name in ("q", "k", "v"):
        w_aps[name] = nc.dram_tensor(f"w{name}", (H, HD, HD), F32, kind="ExternalInput").ap()
        b_aps[name] = nc.dram_tensor(f"b{name}", (H, HD), F32, kind="ExternalInput").ap()
    out = nc.dram_tensor("out", (S, C * DIM), F32, kind="ExternalOutput").ap()

    with tile.TileContext(nc) as tc:
        with contextlib.ExitStack() as ctx:
            _kernel_body(ctx, tc, out, x, w_aps, b_aps)
    nc.compile()
    return nc


def run(inputs, trace=False, **kw):
    le = np.asarray(inputs["linear_embed"], dtype=np.float32)
    nc = build_module()
    in_maps = []
    for b in range(B):
        m = {"x": np.ascontiguousarray(le[b])}
        for name in ("q", "k", "v"):
            m[f"w{name}"] = np.asarray(inputs[f"W{name}"], dtype=np.float32)
            m[f"b{name}"] = np.asarray(inputs[f"b{name}"], dtype=np.float32)
        in_maps.append(m)
    res = bass_utils.run_bass_kernel_spmd(
        nc, in_maps, core_ids=list(range(B)), trace=trace, **kw
    )
    out = np.stack([res.results[b]["out"] for b in range(B)], axis=0)
    return out, res


def kernel(**inputs) -> np.ndarray:
    out, _ = run(inputs)
    return out
